# revision 33
# baseline (speedup 1.0000x reference)
"""DINOv2 self-attention (QKV projection + SDPA, no out-proj) on 8 Trainium2
NeuronCores.

Sharding: pure data-parallel over batch (B=8 -> one batch element per core);
no cross-core communication.

Design (interleaved projections; f32r GEMMs, bf16 attention operands):
  * Projections (x@W) in float32r: self-loading matmuls, full PE rate at
    moving>=256.  Attention operands (q/k tiles, v tiles, exp weights)
    in bf16: measured on HW, the bf16 LDWEIGHTS+MATMUL lowering streams
    better than f32r's serial self-load for the attention shapes, and
    lets the two heads' K=64 score matmuls overlap in disjoint PE
    row-groups (partitions 0/64).  (All-fp16 was measured ~40% SLOWER
    than all-f32r - keep the projections f32r.)
  * Head-pair-outer attention with JUST-IN-TIME projections: q/k live in
    per-head-pair [128, S] bf16 tiles and v in per-quarter [128, NT, 260]
    bf16 tiles from small rotating pools, so xT (f32r, 43.8KB/partition)
    stays resident while attention runs.  Remaining projection work is
    drained into the PE queue a few hundred ns per kt-iteration
    (cost-budgeted), with hard finish() barriers before first use (Tile
    deps follow emission order, so barriers are a correctness matter).
  * Softmax denominator fused into ctx via a ones-column per head
    (v tile col h*65+64 = 1); exp fused with the 1/sqrt(hd) scale; no
    max-subtraction (|scores/8| <= ~6).
  * W host-packed as [q_o|k_o] 256-col blocks per o-tile; all DMAs move
    >=512B lines (below that the DMA engines halve throughput); weights
    prefetched well ahead (HWDGE desc-gen is ~0.6us per dma_start); PE
    warmed with junk transposes during the prologue DMA wait (HAM cold
    throttle); ACT exp table primed in the prologue.
"""

import numpy as np
from contextlib import ExitStack

import concourse.bass as bass
import concourse.bacc as bacc
import concourse.tile as tile
from concourse import mybir
from concourse import bass_utils
from concourse.masks import make_identity

S, D, H, HD = 1370, 1024, 16, 64
F32 = mybir.dt.float32
F32R = mybir.dt.float32r
BF16 = mybir.dt.bfloat16
ND = D // 128                      # 8 contraction tiles
NO = D // 128                      # 8 output tiles per projection
NT = (S + 127) // 128              # 11 token tiles
TSZ = [min(128, S - i * 128) for i in range(NT)]
CHUNKS = [(0, 512), (512, 512), (1024, S - 1024)]
EXP = mybir.ActivationFunctionType.Exp
# deferred-work PE budget (ns) drained per kt-iteration, by chunk width
BUDGET = {512: 420.0, 346: 330.0}
MMNS = 0.4167                      # ns per moving row at 2.4 GHz


def _body(tc, xT, wT, bT, bvb, out, reps=1):
    nc = tc.nc
    with ExitStack() as ctx:
        const = ctx.enter_context(tc.tile_pool(name="const", bufs=1))
        ident = const.tile([65, 65], BF16)
        make_identity(nc, ident)
        bT_sb = const.tile([128, 24], F32)
        nc.sync.dma_start(bT_sb[:], bT[:])
        bvb_sb = const.tile([128, D], F32)
        scratch = const.tile([1, 1], F32)
        # prime the ACT exp table so the 1.3us load hides in the prologue
        nc.scalar.activation(scratch[:], bT_sb[0:1, 0:1], EXP)
        big = ctx.enter_context(tc.tile_pool(name="big", bufs=1))
        state = {"first": True}
        for _rep in range(reps):
            _one_pass(tc, big, ident, bT_sb, bvb_sb, bvb, xT, wT, out, state)


def _one_pass(tc, big, ident, bT_sb, bvb_sb, bvb, xT, wT, out, state):
    nc = tc.nc
    xt = big.tile([128, ND, S], F32R, tag="xt", name="xt")

    with ExitStack() as s:
        wv_pool = s.enter_context(tc.tile_pool(name="wv", bufs=2))
        wqk_pool = s.enter_context(tc.tile_pool(name="wqk", bufs=3))
        qk_pool = s.enter_context(tc.tile_pool(name="qk", bufs=6))
        v_pool = s.enter_context(tc.tile_pool(name="vq", bufs=3))
        et_pool = s.enter_context(tc.tile_pool(name="et", bufs=6))
        cs_pool = s.enter_context(tc.tile_pool(name="cs", bufs=4))
        os_pool = s.enter_context(tc.tile_pool(name="os", bufs=8))
        rec_pool = s.enter_context(tc.tile_pool(name="rec", bufs=8))
        pss = s.enter_context(tc.tile_pool(name="pss", bufs=2, space="PSUM"))
        psc = s.enter_context(tc.tile_pool(name="psc", bufs=1, space="PSUM"))
        psp = s.enter_context(tc.tile_pool(name="psp", bufs=2, space="PSUM"))

        # x loads chunk-outer (sync queue); first weight tiles early on the
        # scalar queue.  Transfers drain roughly in issue order, so the
        # prologue's needs (w-qk0, xt-c0, w-v0) lead.
        wqk_tiles = {}
        wv_tiles = {}

        def _wdma(dst, col):
            nc.scalar.dma_start(
                dst[:], wT[:, col:col + 256].rearrange("(d p) j -> p d j",
                                                       p=128))

        def _load_wqk(o):
            w = wqk_pool.tile([128, ND, 256], F32R, tag="wqk", name=f"wqk{o}")
            _wdma(w, o * 256)
            wqk_tiles[o] = w

        def _load_wv(q):
            w = wv_pool.tile([128, ND, 256], F32R, tag="wv", name=f"wv{q}")
            _wdma(w, 2 * D + q * 256)
            wv_tiles[q] = w

        _load_wqk(0)
        nc.sync.dma_start(
            xt[:, :, 0:512], xT[:, 0:512].rearrange("(d p) j -> p d j",
                                                    p=128))
        _load_wv(0)
        for (c0, cw) in CHUNKS[1:]:
            nc.sync.dma_start(
                xt[:, :, c0:c0 + cw],
                xT[:, c0:c0 + cw].rearrange("(d p) j -> p d j", p=128))
        if state["first"]:
            state["first"] = False
            nc.scalar.dma_start(bvb_sb[:], bvb[:])
        _load_wqk(1)
        _load_wv(1)

        # warm the PE clock (HAM lifts the 1.2GHz cold throttle only after
        # ~3.4us of activity) with junk transposes while the DMAs land
        warm = psp.tile([128, 512], BF16, tag="psp", name="warm")
        for _ in range(140):
            nc.tensor.transpose(warm[:65, :65], ident[:65, :65],
                                ident[:65, :65])

        # ---- projection generators yielding (pe_ns, op) ----
        qk_tiles = {}           # (proj, o) -> [128, S] tile
        v_tiles = {}            # quarter -> [128, NT, 260] tile

        def gen_v_quarter(q):
            """v for heads 4q..4q+3 -> v_tiles[q] [128, NT, 4*65], with a
            ones column per head for the fused softmax denominator."""
            wv = wv_tiles[q]
            vt = v_pool.tile([128, NT, 4 * 65], BF16, tag="vq",
                             name=f"v{q}")
            v_tiles[q] = vt

            def ones():
                for t in range(NT):
                    col = vt[:, t, :].rearrange(
                        "p (h e) -> p h e", e=65)[:, :, 64]
                    nc.vector.tensor_scalar(
                        col, bT_sb[:, 0:4], 0.0, 1.0,
                        mybir.AluOpType.mult, mybir.AluOpType.add)
            yield 0.0, ones
            for t in range(NT):
                tsz = TSZ[t]
                ps = psp.tile([128, 512], F32, tag="psp", name="psv")
                for d in range(ND):
                    def mm(t=t, d=d, ps=ps, tsz=tsz, wv=wv):
                        nc.tensor.matmul(
                            ps[:tsz, :256], xt[:, d, t * 128:t * 128 + tsz],
                            wv[:, d, :], start=(d == 0), stop=(d == ND - 1))
                    yield 256 * MMNS, mm

                def evac(t=t, ps=ps, tsz=tsz, vt=vt):
                    dst = vt[:tsz, t, :].rearrange(
                        "p (h e) -> p h e", e=65)[:, :, 0:64]
                    src = ps[:tsz, :256].rearrange("p (h e) -> p h e", e=64)
                    bias = bvb_sb[:tsz, q * 256:(q + 1) * 256].rearrange(
                        "p (h e) -> p h e", e=64)
                    nc.vector.tensor_add(dst, src, bias)
                yield 0.0, evac

        def gen_qk_o(o, projs=(1, 0), chunks=(0, 1, 2)):
            """q/k projections for o-tile o (heads 2o, 2o+1) into per-pair
            [128, S] tiles.  W is packed [q_o | k_o] per o."""
            w = wqk_tiles[o]
            for proj in projs:
                if (proj, o) not in qk_tiles:
                    qk_tiles[(proj, o)] = qk_pool.tile(
                        [128, S], BF16, tag="qk",
                        name=f"{'qk'[proj]}T{o}")
                dstT = qk_tiles[(proj, o)]
                for ci in chunks:
                    c0, cw = CHUNKS[ci]
                    ps = psp.tile([128, 512], F32, tag="psp", name="psqk")
                    for d in range(ND):
                        def mm(proj=proj, d=d, ps=ps, c0=c0, cw=cw, w=w):
                            nc.tensor.matmul(
                                ps[:, :cw],
                                w[:, d, proj * 128:(proj + 1) * 128],
                                xt[:, d, c0:c0 + cw],
                                start=(d == 0), stop=(d == ND - 1))
                        yield cw * MMNS, mm

                    def evac(proj=proj, ps=ps, c0=c0, cw=cw, o=o,
                             dstT=dstT):
                        nc.vector.tensor_scalar_add(
                            dstT[:, c0:c0 + cw], ps[:, :cw],
                            bT_sb[:, proj * 8 + o:proj * 8 + o + 1])
                    yield 0.0, evac

        # ---- prologue: k/q for head-pair 0, v quarter 0 ----
        for _, op in gen_qk_o(0, projs=(1,), chunks=(0,)):
            op()
        for _, op in gen_qk_o(0, projs=(0,), chunks=(0,)):
            op()
        for _, op in gen_v_quarter(0):       # heads 0-3, fully in prologue
            op()
        for _, op in gen_qk_o(0, projs=(1,), chunks=(1, 2)):
            op()

        # Deferred projection work.  finish(name) is a hard barrier:
        # everything up to and including that generator is EMITTED before
        # the first instruction that reads its outputs.
        class Work:
            def __init__(self, items):
                self.items = list(items)
                self.idx = 0
                self.done = set()

            def drain(self, budget):
                while self.idx < len(self.items):
                    nxt = next(self.items[self.idx][1], None)
                    if nxt is None:
                        self.done.add(self.items[self.idx][0])
                        self.idx += 1
                        continue
                    cost, op = nxt
                    op()
                    budget -= cost
                    if budget <= 0:
                        return

            def finish(self, name):
                if name in self.done:
                    return
                while self.idx < len(self.items):
                    nm, g = self.items[self.idx]
                    for _, op in g:
                        op()
                    self.done.add(nm)
                    self.idx += 1
                    if nm == name:
                        return

        def gen_wdma(fn, arg):
            def op():
                fn(arg)
            yield 0.0, op

        work = Work([
            ("q0c1", gen_qk_o(0, projs=(0,), chunks=(1,))),
            ("q0c2", gen_qk_o(0, projs=(0,), chunks=(2,))),
            ("w2", gen_wdma(_load_wqk, 2)),
            ("o1", gen_qk_o(1)),
            ("w3", gen_wdma(_load_wqk, 3)),
            ("o2", gen_qk_o(2)),
            ("wv2", gen_wdma(_load_wv, 2)),
            ("vq1", gen_v_quarter(1)),               # heads 4-7, by hp2
            ("w4", gen_wdma(_load_wqk, 4)),
            ("o3", gen_qk_o(3)),
            ("w5", gen_wdma(_load_wqk, 5)),
            ("o4", gen_qk_o(4)),
            ("wv3", gen_wdma(_load_wv, 3)),
            ("vq2", gen_v_quarter(2)),               # heads 8-11, by hp4
            ("w6", gen_wdma(_load_wqk, 6)),
            ("o5", gen_qk_o(5)),
            ("w7", gen_wdma(_load_wqk, 7)),
            ("o6", gen_qk_o(6)),
            ("vq3", gen_v_quarter(3)),               # heads 12-15, by hp6
            ("o7", gen_qk_o(7)),
        ])
        BARRIER = {1: "o1", 2: "vq1", 3: "o3", 4: "vq2", 5: "o5",
                   6: "vq3", 7: "o7"}
        import os as _os
        if _os.environ.get("NO_INTERLEAVE"):
            work.finish(None)

        # ---- attention, head-pair outer ----

        def emit_ctx(pcs, ets, hp, kt, cw):
            ksz = TSZ[kt]
            et = ets.pop(kt)
            vt = v_tiles[hp // 2]
            for hi in range(2):
                hl = (hp % 2) * 2 + hi
                nc.tensor.matmul(
                    pcs[:, hi, :cw],
                    vt[:ksz, kt, hl * 65:(hl + 1) * 65],
                    et[:ksz, hi, :cw],
                    start=(kt == 0), stop=(kt == NT - 1))

        def flush(hp, c0, cw, csts):
            sub = [(s0, min(128, cw - s0)) for s0 in range(0, cw, 128)]
            oss = [os_pool.tile([128, 128], F32, tag="os", name="os")
                   for _ in sub]
            for hi, cst in enumerate(csts):
                for si, (s0, ssz) in enumerate(sub):
                    tp = psp.tile([128, 65], BF16, tag="psp", name="tp")
                    nc.tensor.transpose(
                        tp[:ssz, :], cst[:65, s0:s0 + ssz], ident[:65, :65])
                    rec = rec_pool.tile([128, 1], F32, tag="rec", name="rec")
                    nc.vector.reciprocal(rec[:ssz], tp[:ssz, 64:65])
                    nc.vector.tensor_scalar_mul(
                        oss[si][:ssz, hi * 64:(hi + 1) * 64],
                        tp[:ssz, 0:64], rec[:ssz])
            for si, (s0, ssz) in enumerate(sub):
                nc.scalar.dma_start(
                    out[c0 + s0:c0 + s0 + ssz, hp * 128:(hp + 1) * 128],
                    oss[si][:ssz, :])

        for hp in range(8):
            if hp in BARRIER:
                work.finish(BARRIER[hp])
            kTt = qk_tiles[(1, hp)]
            qTt = qk_tiles[(0, hp)]
            for ci, (c0, cw) in enumerate(CHUNKS):
                if hp == 0 and ci >= 1:
                    work.finish(f"q0c{ci}")
                pcs = psc.tile([65, 2, 512], F32, tag="psc", name="psc")
                ets = {}
                for kt in range(NT):
                    k0, ksz = kt * 128, TSZ[kt]
                    ps_s = pss.tile([128, 2, 512], F32, tag="pss", name="pss")
                    for hi in range(2):
                        p0 = hi * 64
                        nc.tensor.matmul(
                            ps_s[:ksz, hi, :cw],
                            kTt[p0:p0 + 64, k0:k0 + ksz],
                            qTt[p0:p0 + 64, c0:c0 + cw],
                            start=True, stop=True)
                    if kt >= 1:
                        emit_ctx(pcs, ets, hp, kt - 1, cw)
                    et = et_pool.tile([128, 2, 512], BF16, tag="et",
                                      name="et")
                    ets[kt] = et
                    nc.scalar.activation(
                        et[:ksz, :, :cw], ps_s[:ksz, :, :cw], EXP, scale=0.125)
                    work.drain(BUDGET[cw])
                emit_ctx(pcs, ets, hp, NT - 1, cw)
                csts = []
                for hi in range(2):
                    cst = cs_pool.tile([65, 512], BF16, tag="cs", name="cs")
                    nc.vector.tensor_copy(cst[:, :cw], pcs[:, hi, :cw])
                    csts.append(cst)
                flush(hp, c0, cw, csts)
        work.finish(None)


def build_program(reps=1):
    nc = bacc.Bacc("TRN2", target_bir_lowering=False, debug=False,
                   num_devices=8)
    xT = nc.dram_tensor("xT", [D, S], F32R, kind="ExternalInput").ap()
    wT = nc.dram_tensor("wT", [D, 3 * D], F32R, kind="ExternalInput").ap()
    bT = nc.dram_tensor("bT", [128, 24], F32, kind="ExternalInput").ap()
    bvb = nc.dram_tensor("bvb", [128, D], F32, kind="ExternalInput").ap()
    out = nc.dram_tensor("out", [S, D], F32, kind="ExternalOutput").ap()
    with tile.TileContext(nc) as tc:
        _body(tc, xT, wT, bT, bvb, out, reps=reps)
    nc.compile()
    return nc


_PROGRAM = None


def _get_program():
    global _PROGRAM
    if _PROGRAM is None:
        _PROGRAM = build_program()
    return _PROGRAM


def _prep_inputs(hidden_states, Wq, bq, Wk, bk, Wv, bv):
    hs = np.asarray(hidden_states, dtype=np.float32)
    B = hs.shape[0]
    xT = np.ascontiguousarray(hs.transpose(0, 2, 1))
    WqT = np.asarray(Wq, dtype=np.float32).T
    WkT = np.asarray(Wk, dtype=np.float32).T
    WvT = np.asarray(Wv, dtype=np.float32).T
    # pack q|k per o-tile in 256-col blocks, v appended at col 2048
    qk = np.empty((D, 2 * D), dtype=np.float32)
    for o in range(NO):
        qk[:, o * 256:o * 256 + 128] = WqT[:, o * 128:(o + 1) * 128]
        qk[:, o * 256 + 128:(o + 1) * 256] = WkT[:, o * 128:(o + 1) * 128]
    wT = np.ascontiguousarray(np.concatenate([qk, WvT], axis=1))
    b_all = np.concatenate([np.asarray(bq, dtype=np.float32),
                            np.asarray(bk, dtype=np.float32),
                            np.asarray(bv, dtype=np.float32)])
    bT_np = np.ascontiguousarray(b_all.reshape(24, 128).T)
    bvb_np = np.ascontiguousarray(
        np.broadcast_to(np.asarray(bv, dtype=np.float32), (128, D)))
    return [{"xT": xT[b], "wT": wT, "bT": bT_np, "bvb": bvb_np}
            for b in range(B)]


def run(in_maps, **kw):
    nc = _get_program()
    return bass_utils.run_bass_kernel_spmd(
        nc, in_maps, core_ids=list(range(len(in_maps))), **kw)


def kernel(hidden_states, Wq, bq, Wk, bk, Wv, bv):
    in_maps = _prep_inputs(hidden_states, Wq, bq, Wk, bk, Wv, bv)
    res = run(in_maps)
    return np.stack([res.results[b]["out"] for b in range(len(in_maps))],
                    axis=0)


# revision 36
# speedup vs baseline: 1.0693x; 1.0693x over previous
"""DINOv2 self-attention (QKV projection + SDPA, no out-proj) on 8 Trainium2
NeuronCores.

Sharding: pure data-parallel over batch (B=8 -> one batch element per core);
no cross-core communication.

Design (interleaved projections; all matmul operands bf16, f32 PSUM):
  * Every matmul operand (x, W, q/k tiles, v tiles, exp weights) is bf16:
    measured on HW via paired A/B, the bf16 LDWEIGHTS+MATMUL lowering
    streams better than f32r's serial internal weight load for every
    matmul class here, and lets the two heads' K=64 score matmuls overlap
    in disjoint PE row-groups (partitions 0/64).  Accumulation is f32 in
    PSUM; biases, softmax denominators and the output stay f32.
  * Head-pair-outer attention with JUST-IN-TIME projections: q/k live in
    per-head-pair [128, S] bf16 tiles and v in per-quarter [128, NT, 260]
    bf16 tiles from small rotating pools, so xT (bf16, 21.9KB/partition)
    stays resident while attention runs.  Remaining projection work is
    drained into the PE queue a few hundred ns per kt-iteration
    (cost-budgeted), with hard finish() barriers before first use (Tile
    deps follow emission order, so barriers are a correctness matter).
  * Softmax denominator fused into ctx via a ones-column per head
    (v tile col h*65+64 = 1); exp fused with the 1/sqrt(hd) scale; no
    max-subtraction (|scores/8| <= ~6).
  * W host-packed as [q_o|k_o] 256-col blocks per o-tile; all DMAs move
    >=512B lines (below that the DMA engines halve throughput); weights
    prefetched well ahead (HWDGE desc-gen is ~0.6us per dma_start); PE
    warmed with junk transposes during the prologue DMA wait (HAM cold
    throttle); ACT exp table primed in the prologue.
"""

import numpy as np
import ml_dtypes
BF16NP = ml_dtypes.bfloat16
from contextlib import ExitStack

import concourse.bass as bass
import concourse.bacc as bacc
import concourse.tile as tile
from concourse import mybir
from concourse import bass_utils
from concourse.masks import make_identity

S, D, H, HD = 1370, 1024, 16, 64
F32 = mybir.dt.float32
F32R = mybir.dt.float32r
BF16 = mybir.dt.bfloat16
ND = D // 128                      # 8 contraction tiles
NO = D // 128                      # 8 output tiles per projection
NT = (S + 127) // 128              # 11 token tiles
TSZ = [min(128, S - i * 128) for i in range(NT)]
CHUNKS = [(0, 512), (512, 512), (1024, S - 1024)]
EXP = mybir.ActivationFunctionType.Exp
# deferred-work PE budget (ns) drained per kt-iteration, by chunk width
BUDGET = {512: 420.0, 346: 330.0}
MMNS = 0.4167                      # ns per moving row at 2.4 GHz


def _body(tc, xT, wT, bT, bvb, out, reps=1):
    nc = tc.nc
    with ExitStack() as ctx:
        const = ctx.enter_context(tc.tile_pool(name="const", bufs=1))
        ident = const.tile([65, 65], BF16)
        make_identity(nc, ident)
        bT_sb = const.tile([128, 24], F32)
        nc.sync.dma_start(bT_sb[:], bT[:])
        bvb_sb = const.tile([128, D], F32)
        scratch = const.tile([1, 1], F32)
        # prime the ACT exp table so the 1.3us load hides in the prologue
        nc.scalar.activation(scratch[:], bT_sb[0:1, 0:1], EXP)
        big = ctx.enter_context(tc.tile_pool(name="big", bufs=1))
        state = {"first": True}
        for _rep in range(reps):
            _one_pass(tc, big, ident, bT_sb, bvb_sb, bvb, xT, wT, out, state)


def _one_pass(tc, big, ident, bT_sb, bvb_sb, bvb, xT, wT, out, state):
    nc = tc.nc
    xt = big.tile([128, ND, S], BF16, tag="xt", name="xt")

    with ExitStack() as s:
        wv_pool = s.enter_context(tc.tile_pool(name="wv", bufs=2))
        wqk_pool = s.enter_context(tc.tile_pool(name="wqk", bufs=3))
        qk_pool = s.enter_context(tc.tile_pool(name="qk", bufs=6))
        v_pool = s.enter_context(tc.tile_pool(name="vq", bufs=3))
        et_pool = s.enter_context(tc.tile_pool(name="et", bufs=8))
        cs_pool = s.enter_context(tc.tile_pool(name="cs", bufs=6))
        os_pool = s.enter_context(tc.tile_pool(name="os", bufs=10))
        rec_pool = s.enter_context(tc.tile_pool(name="rec", bufs=8))
        pss = s.enter_context(tc.tile_pool(name="pss", bufs=2, space="PSUM"))
        psc = s.enter_context(tc.tile_pool(name="psc", bufs=1, space="PSUM"))
        psp = s.enter_context(tc.tile_pool(name="psp", bufs=2, space="PSUM"))

        # x loads chunk-outer (sync queue); first weight tiles early on the
        # scalar queue.  Transfers drain roughly in issue order, so the
        # prologue's needs (w-qk0, xt-c0, w-v0) lead.
        wqk_tiles = {}
        wv_tiles = {}

        def _wdma(dst, col):
            nc.scalar.dma_start(
                dst[:], wT[:, col:col + 256].rearrange("(d p) j -> p d j",
                                                       p=128))

        def _load_wqk(o):
            w = wqk_pool.tile([128, ND, 256], BF16, tag="wqk", name=f"wqk{o}")
            _wdma(w, o * 256)
            wqk_tiles[o] = w

        def _load_wv(q):
            w = wv_pool.tile([128, ND, 256], BF16, tag="wv", name=f"wv{q}")
            _wdma(w, 2 * D + q * 256)
            wv_tiles[q] = w

        _load_wqk(0)
        nc.sync.dma_start(
            xt[:, :, 0:512], xT[:, 0:512].rearrange("(d p) j -> p d j",
                                                    p=128))
        _load_wv(0)
        for (c0, cw) in CHUNKS[1:]:
            nc.sync.dma_start(
                xt[:, :, c0:c0 + cw],
                xT[:, c0:c0 + cw].rearrange("(d p) j -> p d j", p=128))
        if state["first"]:
            state["first"] = False
            nc.scalar.dma_start(bvb_sb[:], bvb[:])
        _load_wqk(1)
        _load_wv(1)

        # warm the PE clock (HAM lifts the 1.2GHz cold throttle only after
        # ~3.4us of activity) with junk transposes while the DMAs land
        warm = psp.tile([128, 512], BF16, tag="psp", name="warm")
        for _ in range(140):
            nc.tensor.transpose(warm[:65, :65], ident[:65, :65],
                                ident[:65, :65])

        # ---- projection generators yielding (pe_ns, op) ----
        qk_tiles = {}           # (proj, o) -> [128, S] tile
        v_tiles = {}            # quarter -> [128, NT, 260] tile

        def gen_v_quarter(q):
            """v for heads 4q..4q+3 -> v_tiles[q] [128, NT, 4*65], with a
            ones column per head for the fused softmax denominator."""
            wv = wv_tiles[q]
            vt = v_pool.tile([128, NT, 4 * 65], BF16, tag="vq",
                             name=f"v{q}")
            v_tiles[q] = vt

            def ones():
                for t in range(NT):
                    col = vt[:, t, :].rearrange(
                        "p (h e) -> p h e", e=65)[:, :, 64]
                    nc.vector.tensor_scalar(
                        col, bT_sb[:, 0:4], 0.0, 1.0,
                        mybir.AluOpType.mult, mybir.AluOpType.add)
            yield 0.0, ones
            for t in range(NT):
                tsz = TSZ[t]
                ps = psp.tile([128, 512], F32, tag="psp", name="psv")
                for d in range(ND):
                    def mm(t=t, d=d, ps=ps, tsz=tsz, wv=wv):
                        nc.tensor.matmul(
                            ps[:tsz, :256], xt[:, d, t * 128:t * 128 + tsz],
                            wv[:, d, :], start=(d == 0), stop=(d == ND - 1))
                    yield 256 * MMNS, mm

                def evac(t=t, ps=ps, tsz=tsz, vt=vt):
                    dst = vt[:tsz, t, :].rearrange(
                        "p (h e) -> p h e", e=65)[:, :, 0:64]
                    src = ps[:tsz, :256].rearrange("p (h e) -> p h e", e=64)
                    bias = bvb_sb[:tsz, q * 256:(q + 1) * 256].rearrange(
                        "p (h e) -> p h e", e=64)
                    nc.vector.tensor_add(dst, src, bias)
                yield 0.0, evac

        def gen_qk_o(o, projs=(1, 0), chunks=(0, 1, 2)):
            """q/k projections for o-tile o (heads 2o, 2o+1) into per-pair
            [128, S] tiles.  W is packed [q_o | k_o] per o."""
            w = wqk_tiles[o]
            for proj in projs:
                if (proj, o) not in qk_tiles:
                    qk_tiles[(proj, o)] = qk_pool.tile(
                        [128, S], BF16, tag="qk",
                        name=f"{'qk'[proj]}T{o}")
                dstT = qk_tiles[(proj, o)]
                for ci in chunks:
                    c0, cw = CHUNKS[ci]
                    ps = psp.tile([128, 512], F32, tag="psp", name="psqk")
                    for d in range(ND):
                        def mm(proj=proj, d=d, ps=ps, c0=c0, cw=cw, w=w):
                            nc.tensor.matmul(
                                ps[:, :cw],
                                w[:, d, proj * 128:(proj + 1) * 128],
                                xt[:, d, c0:c0 + cw],
                                start=(d == 0), stop=(d == ND - 1))
                        yield cw * MMNS, mm

                    def evac(proj=proj, ps=ps, c0=c0, cw=cw, o=o,
                             dstT=dstT):
                        nc.vector.tensor_scalar_add(
                            dstT[:, c0:c0 + cw], ps[:, :cw],
                            bT_sb[:, proj * 8 + o:proj * 8 + o + 1])
                    yield 0.0, evac

        # ---- prologue: k/q for head-pair 0, v quarter 0 ----
        for _, op in gen_qk_o(0, projs=(1,), chunks=(0,)):
            op()
        for _, op in gen_qk_o(0, projs=(0,), chunks=(0,)):
            op()
        for _, op in gen_v_quarter(0):       # heads 0-3, fully in prologue
            op()
        for _, op in gen_qk_o(0, projs=(1,), chunks=(1, 2)):
            op()

        # Deferred projection work.  finish(name) is a hard barrier:
        # everything up to and including that generator is EMITTED before
        # the first instruction that reads its outputs.
        class Work:
            def __init__(self, items):
                self.items = list(items)
                self.idx = 0
                self.done = set()

            def drain(self, budget):
                while self.idx < len(self.items):
                    nxt = next(self.items[self.idx][1], None)
                    if nxt is None:
                        self.done.add(self.items[self.idx][0])
                        self.idx += 1
                        continue
                    cost, op = nxt
                    op()
                    budget -= cost
                    if budget <= 0:
                        return

            def finish(self, name):
                if name in self.done:
                    return
                while self.idx < len(self.items):
                    nm, g = self.items[self.idx]
                    for _, op in g:
                        op()
                    self.done.add(nm)
                    self.idx += 1
                    if nm == name:
                        return

        def gen_wdma(fn, arg):
            def op():
                fn(arg)
            yield 0.0, op

        work = Work([
            ("q0c1", gen_qk_o(0, projs=(0,), chunks=(1,))),
            ("q0c2", gen_qk_o(0, projs=(0,), chunks=(2,))),
            ("w2", gen_wdma(_load_wqk, 2)),
            ("o1", gen_qk_o(1)),
            ("w3", gen_wdma(_load_wqk, 3)),
            ("o2", gen_qk_o(2)),
            ("wv2", gen_wdma(_load_wv, 2)),
            ("vq1", gen_v_quarter(1)),               # heads 4-7, by hp2
            ("w4", gen_wdma(_load_wqk, 4)),
            ("o3", gen_qk_o(3)),
            ("w5", gen_wdma(_load_wqk, 5)),
            ("o4", gen_qk_o(4)),
            ("wv3", gen_wdma(_load_wv, 3)),
            ("vq2", gen_v_quarter(2)),               # heads 8-11, by hp4
            ("w6", gen_wdma(_load_wqk, 6)),
            ("o5", gen_qk_o(5)),
            ("w7", gen_wdma(_load_wqk, 7)),
            ("o6", gen_qk_o(6)),
            ("vq3", gen_v_quarter(3)),               # heads 12-15, by hp6
            ("o7", gen_qk_o(7)),
        ])
        BARRIER = {1: "o1", 2: "vq1", 3: "o3", 4: "vq2", 5: "o5",
                   6: "vq3", 7: "o7"}
        import os as _os
        if _os.environ.get("NO_INTERLEAVE"):
            work.finish(None)

        # ---- attention, head-pair outer ----

        def emit_ctx(pcs, ets, hp, kt, cw):
            ksz = TSZ[kt]
            et = ets.pop(kt)
            vt = v_tiles[hp // 2]
            for hi in range(2):
                hl = (hp % 2) * 2 + hi
                nc.tensor.matmul(
                    pcs[:, hi, :cw],
                    vt[:ksz, kt, hl * 65:(hl + 1) * 65],
                    et[:ksz, hi, :cw],
                    start=(kt == 0), stop=(kt == NT - 1))

        def flush(hp, c0, cw, csts):
            sub = [(s0, min(128, cw - s0)) for s0 in range(0, cw, 128)]
            oss = [os_pool.tile([128, 128], F32, tag="os", name="os")
                   for _ in sub]
            for hi, cst in enumerate(csts):
                for si, (s0, ssz) in enumerate(sub):
                    tp = psp.tile([128, 65], BF16, tag="psp", name="tp")
                    nc.tensor.transpose(
                        tp[:ssz, :], cst[:65, s0:s0 + ssz], ident[:65, :65])
                    rec = rec_pool.tile([128, 1], F32, tag="rec", name="rec")
                    nc.vector.reciprocal(rec[:ssz], tp[:ssz, 64:65])
                    nc.vector.tensor_scalar_mul(
                        oss[si][:ssz, hi * 64:(hi + 1) * 64],
                        tp[:ssz, 0:64], rec[:ssz])
            for si, (s0, ssz) in enumerate(sub):
                nc.scalar.dma_start(
                    out[c0 + s0:c0 + s0 + ssz, hp * 128:(hp + 1) * 128],
                    oss[si][:ssz, :])

        for hp in range(8):
            if hp in BARRIER:
                work.finish(BARRIER[hp])
            kTt = qk_tiles[(1, hp)]
            qTt = qk_tiles[(0, hp)]
            for ci, (c0, cw) in enumerate(CHUNKS):
                if hp == 0 and ci >= 1:
                    work.finish(f"q0c{ci}")
                pcs = psc.tile([65, 2, 512], F32, tag="psc", name="psc")
                ets = {}
                for kt in range(NT):
                    k0, ksz = kt * 128, TSZ[kt]
                    ps_s = pss.tile([128, 2, 512], F32, tag="pss", name="pss")
                    for hi in range(2):
                        p0 = hi * 64
                        nc.tensor.matmul(
                            ps_s[:ksz, hi, :cw],
                            kTt[p0:p0 + 64, k0:k0 + ksz],
                            qTt[p0:p0 + 64, c0:c0 + cw],
                            start=True, stop=True)
                    if kt >= 1:
                        emit_ctx(pcs, ets, hp, kt - 1, cw)
                    et = et_pool.tile([128, 2, 512], BF16, tag="et",
                                      name="et")
                    ets[kt] = et
                    nc.scalar.activation(
                        et[:ksz, :, :cw], ps_s[:ksz, :, :cw], EXP, scale=0.125)
                    work.drain(BUDGET[cw])
                emit_ctx(pcs, ets, hp, NT - 1, cw)
                csts = []
                for hi in range(2):
                    cst = cs_pool.tile([65, 512], BF16, tag="cs", name="cs")
                    nc.vector.tensor_copy(cst[:, :cw], pcs[:, hi, :cw])
                    csts.append(cst)
                flush(hp, c0, cw, csts)
        work.finish(None)


def build_program(reps=1):
    nc = bacc.Bacc("TRN2", target_bir_lowering=False, debug=False,
                   num_devices=8)
    xT = nc.dram_tensor("xT", [D, S], BF16, kind="ExternalInput").ap()
    wT = nc.dram_tensor("wT", [D, 3 * D], BF16, kind="ExternalInput").ap()
    bT = nc.dram_tensor("bT", [128, 24], F32, kind="ExternalInput").ap()
    bvb = nc.dram_tensor("bvb", [128, D], F32, kind="ExternalInput").ap()
    out = nc.dram_tensor("out", [S, D], F32, kind="ExternalOutput").ap()
    with tile.TileContext(nc) as tc:
        _body(tc, xT, wT, bT, bvb, out, reps=reps)
    nc.compile()
    return nc


_PROGRAM = None


def _get_program():
    global _PROGRAM
    if _PROGRAM is None:
        _PROGRAM = build_program()
    return _PROGRAM


def _prep_inputs(hidden_states, Wq, bq, Wk, bk, Wv, bv):
    hs = np.asarray(hidden_states, dtype=np.float32)
    B = hs.shape[0]
    xT = np.ascontiguousarray(hs.transpose(0, 2, 1)).astype(BF16NP)
    WqT = np.asarray(Wq, dtype=np.float32).T
    WkT = np.asarray(Wk, dtype=np.float32).T
    WvT = np.asarray(Wv, dtype=np.float32).T
    # pack q|k per o-tile in 256-col blocks, v appended at col 2048
    qk = np.empty((D, 2 * D), dtype=np.float32)
    for o in range(NO):
        qk[:, o * 256:o * 256 + 128] = WqT[:, o * 128:(o + 1) * 128]
        qk[:, o * 256 + 128:(o + 1) * 256] = WkT[:, o * 128:(o + 1) * 128]
    wT = np.ascontiguousarray(np.concatenate([qk, WvT], axis=1)).astype(BF16NP)
    b_all = np.concatenate([np.asarray(bq, dtype=np.float32),
                            np.asarray(bk, dtype=np.float32),
                            np.asarray(bv, dtype=np.float32)])
    bT_np = np.ascontiguousarray(b_all.reshape(24, 128).T)
    bvb_np = np.ascontiguousarray(
        np.broadcast_to(np.asarray(bv, dtype=np.float32), (128, D)))
    return [{"xT": xT[b], "wT": wT, "bT": bT_np, "bvb": bvb_np}
            for b in range(B)]


def run(in_maps, **kw):
    nc = _get_program()
    return bass_utils.run_bass_kernel_spmd(
        nc, in_maps, core_ids=list(range(len(in_maps))), **kw)


def kernel(hidden_states, Wq, bq, Wk, bk, Wv, bv):
    in_maps = _prep_inputs(hidden_states, Wq, bq, Wk, bk, Wv, bv)
    res = run(in_maps)
    return np.stack([res.results[b]["out"] for b in range(len(in_maps))],
                    axis=0)
